# revision 16
# baseline (speedup 1.0000x reference)
"""Multi-head attention (B=2, S=2048, H=16, D=128, fp32, non-causal) on 8
Trainium2 NeuronCores.

Strategy: the 32 (batch, head) pairs are independent -> head-parallel
(Ulysses-style) sharding, 4 pairs per core, no on-device collectives.
The host pre-transposes Q and K to [d, s] layout per pair (so the
contraction dim d lands on SBUF partitions with no on-chip transposes),
and the kernel produces out^T [d, s] which the host transposes back.

Per pair the kernel computes scores^T = K @ Q^T tile-by-tile on the PE
(softmax's reduction dim sk lands on partitions). Q is pre-scaled by
softmax_scale/16 on the host so the PSUM scores arrive as t = s*scale/16;
exp(16t) is then computed on TWO engines in parallel:
  - ACT: activation Exp with scale=16 (most groups)
  - DVE: two custom ops: p = deg-4 minimax poly(t) ~ e^t, then p^16 via
    four squarings (exact to ~9e-4 rel over the full score range)
The per-column exp sums accumulate on the DVE and GPSIMD(Pool) engines
(fp16 2x adds), fold to [128,512], partition-reduce via a ones-matmul,
reciprocal via the 1-instruction DVE RECIPROCAL_APPROX_FAST, and a DVE
multiply normalizes the PV accumulation from PSUM.

The PE instruction stream is a flat software pipeline across all
(pair, qblock, group) work items with PV lagging QK by LAG groups, so
the PE never waits on exp latency and stays in its max p-state.
"""

import math

import numpy as np

B, S, H, D = 2, 2048, 16, 128
N_CORES = 8
PAIRS_PER_CORE = (B * H) // N_CORES  # 4
P = 128
QBLK = 512  # q columns per q-block (one PSUM bank of fp32)
N_QB = S // QBLK  # 4
N_SK = S // P  # 16 sk tiles per pair
SK_PER_GROUP = 2  # sk tiles per scores/exp group ([128, 1024] psum tiles)
N_GROUPS = N_SK // SK_PER_GROUP  # 8
GW = SK_PER_GROUP * QBLK  # group width: 1024
SCALE = 1.0 / math.sqrt(D)

# deg-4 minimax fit of e^t on [-0.62, 0.62] with p(0)=1 pinned, factored as
# m*(u^2+Q1B*u+Q1C)(u^2+Q2B*u+1) in u = GAMMA*t; the unit constant term of
# the second quadratic keeps the DVE op to 3 scalars + the One hw const
# (the Src1-carrying encoding faults on this walrus), and m folds into the
# first squaring of the ^16 op. Max rel err of p^16 vs e^(16t): 1.3e-3
# (scores*scale peak at 8.92 -> t=0.56).
EXP_GAMMA = 0.5040394520163475
EXP_Q1B = 0.2899882226460399
EXP_Q1C = 1.5697056417858954
EXP_Q2B = 1.7986497690988466
EXP_M = 0.6370621174950181
ACT_SCALE = 16.0 / EXP_GAMMA   # scores arrive as s*SCALE*GAMMA/16

LAG = 2  # PV trails QK by this many groups in the PE stream

# exp-engine assignment: DVE takes these groups of each qblock, ACT the rest.
# The GPSIMD(Pool) engine is kept OFF the hot path: its tensor ops run 4-7x
# slower than DVE and stall concurrent DVE ops on the shared SBUF write port
# (measured: DVE adds collide up to 3.6us while Pool runs), and GPSIMD cannot
# access PSUM at all.
DVE_EXP_GROUPS = (0,)

_COMPILED = None
_CUSTOM_OPS = None


def _patch_tile_drain():
    """Workaround for walrus 'Too many sync wait commands' on the TileContext
    tail Drain: redistribute all but one of the drain's sem waits onto
    single-wait NoOps on the sync engine (program order places them after the
    drain and before the all-engine barrier, which preserves semantics)."""
    import concourse.mybir as mybir
    import concourse.tile as tile
    from concourse.vector_clock import ScopedClock

    if getattr(tile.TileContext, "_ant_drain_patched", False):
        return

    def _drain_and_barrier(self, tick_clock, wait_clock):
        drain_inst = self.nc.sync.drain()
        wait_clock.add_sem_waits(
            drain_inst.ins, ScopedClock({None: tick_clock.global_clock})
        )
        si = drain_inst.ins.sync_info
        if si is not None and si.on_wait and len(si.on_wait) > 1:
            waits = list(si.on_wait)
            si.on_wait = waits[:1]
            engines = [
                self.nc.sync, self.nc.vector, self.nc.scalar,
                self.nc.tensor, self.nc.gpsimd,
            ]
            for i, w in enumerate(waits[1:]):
                nop = engines[i % len(engines)].nop(nofuse=True)
                nop.ins.sync_info = mybir.SyncInfo(on_wait=[w], on_update=[])

        self.nc.all_engine_barrier()
        assert self.sems is not None
        popped = self.nc._tile_sem_poison_stack.pop()
        assert popped is self._sem_poison
        self.nc.clear_and_free_semaphores(list(self.sems.allocated().values()))
        self.nc.all_engine_barrier()

    tile.TileContext._drain_and_barrier = _drain_and_barrier
    tile.TileContext._ant_drain_patched = True


def _split_excess_waits(nc):
    """This container's walrus rejects instructions carrying more than a
    struct-dependent number of semaphore waits (setupSyncWait: 'Too many
    sync wait commands'): 1 for Matmult/Ldweights (S3_LW struct), 2 for
    everything else. Hoist the excess onto NoOps inserted just before the
    instruction on the same engine — same-engine program order guarantees
    they are honored before the instruction issues."""
    import concourse.mybir as mybir

    seq = 0
    for f in nc.m.functions:
        for b in f.blocks:
            insts = list(b.instructions)
            out = []
            changed = False
            for inst in insts:
                max_waits = 1
                si = inst.sync_info
                if si is not None and si.on_wait and len(si.on_wait) > max_waits:
                    waits = list(si.on_wait)
                    si.on_wait = waits[:max_waits]
                    for w in waits[max_waits:]:
                        nop = mybir.InstNoOp(name=f"ant-waitsplit-{seq}")
                        seq += 1
                        nop.engine = inst.engine
                        nop.sync_info = mybir.SyncInfo(
                            on_wait=[w], on_update=[]
                        )
                        out.append(nop)
                    changed = True
                out.append(inst)
            if changed:
                b.instructions = out
    return nc


def _get_custom_ops():
    """Register two custom DVE ops at runtime (self-pinning their uop shas):
    EXP16_P4_ANT: p = 1 + t*((c1 + c2*t) + t^2*(c3 + c4*t)), c4 via Src1.
    EXP16_SQ4_ANT: p^16 via four squarings."""
    global _CUSTOM_OPS
    if _CUSTOM_OPS is not None:
        return _CUSTOM_OPS
    import concourse.dve_ops as dve_ops
    from concourse.dve_ops import DveOp
    from concourse.dve_spec import (
        C0, C1, C2, One, Spec, Src0, Src1, _has_src1, lower, sq,
    )
    from concourse.dve_uop import DveOpSpec

    def register(name, spec):
        if name not in dve_ops._SUB_OPCODE_FOR_NAME:
            row = max(dve_ops._SUB_OPCODE_FOR_NAME.values()) + 1
            assert row < 0x20, "custom-DVE opcode rows exhausted"
            dve_ops._SUB_OPCODE_FOR_NAME[name] = row
        row = dve_ops._SUB_OPCODE_FOR_NAME[name]
        uops = lower(spec, ver="v3")
        sha = DveOpSpec(
            name=name, opcode=row, uops=uops, rd1_en=_has_src1(spec)
        ).sha("v3")
        op = DveOp(name, spec, subdim=False, uops_sha={"v3": sha})
        existing = [o for o in dve_ops.OPS if o.name == name]
        if not existing:
            dve_ops.OPS.append(op)
            return op
        return existing[0]

    u = Src0
    u2 = sq(u)
    poly = ((u2 + u * C0) + C1) * ((u2 + u * C2) + One)
    exp_p4 = register(
        "EXP16_P4B_ANT",
        Spec(
            body=poly,
            reference=lambda in0, in1, s0, s1, imm2: (
                in0 * in0 + s0 * in0 + s1
            ) * (in0 * in0 + imm2 * in0 + 1.0),
        ),
    )
    sq4 = register(
        "EXP16_SQ4B_ANT",
        Spec(
            body=sq(sq(sq(sq(Src0 * C0)))),
            reference=lambda in0, in1, s0, s1, imm2: (s0 * in0) ** 16,
        ),
    )
    _CUSTOM_OPS = (exp_p4, sq4)
    return _CUSTOM_OPS


def _build():
    import concourse.bass as bass
    import concourse.mybir as mybir
    import concourse.tile as tile

    _patch_tile_drain()
    exp_p4, sq4 = _get_custom_ops()

    f32 = mybir.dt.float32
    f32r = mybir.dt.float32r
    f16 = mybir.dt.float16
    nc = bass.Bass()

    # Q arrives pre-scaled by SCALE/16 and pre-rounded to the fp32r grid;
    # K pre-rounded to fp32r; V pre-cast to fp16.
    qT = nc.dram_tensor("qT", [PAIRS_PER_CORE, P, S], f32r, kind="ExternalInput")
    kT = nc.dram_tensor("kT", [PAIRS_PER_CORE, P, S], f32r, kind="ExternalInput")
    v = nc.dram_tensor("v", [PAIRS_PER_CORE, S, D], f16, kind="ExternalInput")
    outT = nc.dram_tensor("outT", [PAIRS_PER_CORE, P, S], f32, kind="ExternalOutput")

    TOT = PAIRS_PER_CORE * N_QB * N_GROUPS  # flat group count

    def decode(G):
        return G // (N_QB * N_GROUPS), (G // N_GROUPS) % N_QB, G % N_GROUPS

    with tile.TileContext(nc) as tc:
        with (
            tc.tile_pool(name="const", bufs=1) as const_pool,
            tc.tile_pool(name="inp", bufs=2) as inp_pool,
            tc.tile_pool(name="exp", bufs=8) as exp_pool,
            tc.tile_pool(name="tmp", bufs=2) as tmp_pool,
            tc.tile_pool(name="accd", bufs=2) as accd_pool,
            tc.tile_pool(name="fold", bufs=2) as fold_pool,
            tc.tile_pool(name="outsb", bufs=4) as out_pool,
            tc.tile_pool(name="sc_ps", bufs=2, space="PSUM") as sc_psum,
            tc.tile_pool(name="o_ps", bufs=2, space="PSUM") as o_psum,
            tc.tile_pool(name="s_ps", bufs=2, space="PSUM") as s_psum,
        ):
            ones = const_pool.tile([P, P], f16)
            nc.vector.memset(ones[:], 1.0)

            def emit_loads(pair):
                qT_sb = inp_pool.tile([P, S], f32r, tag="qT")
                kT_sb = inp_pool.tile([P, S], f32r, tag="kT")
                v_sb = inp_pool.tile([P, N_SK, D], f16, tag="v")
                nQ = 4
                sl0 = slice(0, S // nQ)
                # order matters: the DMA queue is FIFO and the first QK only
                # needs kT/qT chunk 0, while the first PV (LAG groups later)
                # needs V -- queue V right after the first chunks
                nc.sync.dma_start(kT_sb[:, sl0], kT[pair][:, sl0])
                nc.sync.dma_start(qT_sb[:, sl0], qT[pair][:, sl0])
                nc.sync.dma_start(
                    v_sb[:], v[pair].rearrange("(t p) d -> p t d", p=P)
                )
                for h in range(1, nQ):
                    sl = slice(h * (S // nQ), (h + 1) * (S // nQ))
                    nc.sync.dma_start(kT_sb[:, sl], kT[pair][:, sl])
                rest = slice(S // nQ, S)
                nc.sync.dma_start(qT_sb[:, rest], qT[pair][:, rest])
                return qT_sb, kT_sb, v_sb

            tiles = {0: emit_loads(0)}
            # per-live-qblock state: (out_ps, accd, accp, e_tiles, sums_ps...)
            state = {}

            for G in range(TOT + LAG):
                if G < TOT:
                    pair, qb, g = decode(G)
                    if g == 0 and qb == 0 and pair + 1 < PAIRS_PER_CORE:
                        tiles[pair + 1] = emit_loads(pair + 1)
                    qT_sb, kT_sb, v_sb = tiles[pair]
                    key = (pair, qb)
                    if g == 0:
                        state[key] = {
                            "out_ps": o_psum.tile(
                                [P, QBLK], f32, tag="ops", name="out_ps"
                            ),
                            "acc": accd_pool.tile(
                                [P, GW], f16, tag="acc", name="acc"
                            ),
                            "e": [None] * N_GROUPS,
                        }
                    st = state[key]
                    q_sl = slice(qb * QBLK, (qb + 1) * QBLK)

                    # scores^T for 2 sk tiles
                    sc = sc_psum.tile([P, GW], f32, tag="sc")
                    for j in range(SK_PER_GROUP):
                        sk = g * SK_PER_GROUP + j
                        nc.tensor.matmul(
                            sc[:, j * QBLK : (j + 1) * QBLK],
                            kT_sb[:, sk * P : (sk + 1) * P],
                            qT_sb[:, q_sl],
                            start=True,
                            stop=True,
                        )
                    # exp on ACT or DVE
                    e = exp_pool.tile([P, GW], f16, tag="e")
                    st["e"][g] = e
                    if g in DVE_EXP_GROUPS:
                        tmp = tmp_pool.tile([P, GW], f32, tag="tmp")
                        nc.vector._custom_dve(
                            exp_p4, out=tmp[:], in0=sc[:],
                            s0=EXP_Q1B, s1=EXP_Q1C, imm2=EXP_Q2B,
                        )
                        nc.vector._custom_dve(sq4, out=e[:], in0=tmp[:], s0=EXP_M)
                    else:
                        nc.scalar.activation(
                            e[:], sc[:], mybir.ActivationFunctionType.Exp,
                            scale=ACT_SCALE,
                        )
                    # exp-sum accumulation on DVE (fp16 2x); the first add
                    # at g=1 combines groups 0 and 1, saving the init copy
                    acc = st["acc"]
                    if g == 1:
                        nc.vector.tensor_add(acc[:], st["e"][0][:], e[:])
                    elif g >= 2:
                        nc.vector.tensor_add(acc[:], acc[:], e[:])

                if G - LAG >= 0:
                    pair2, qb2, g2 = decode(G - LAG)
                    key2 = (pair2, qb2)
                    st2 = state[key2]
                    v_sb2 = tiles[pair2][2]
                    ep = st2["e"][g2]
                    out_ps = st2["out_ps"]
                    for j in range(SK_PER_GROUP):
                        sk = g2 * SK_PER_GROUP + j
                        nc.tensor.matmul(
                            out_ps[:],
                            v_sb2[:, sk, :],
                            ep[:, j * QBLK : (j + 1) * QBLK],
                            start=(sk == 0),
                            stop=(sk == N_SK - 1),
                        )
                    if g2 == N_GROUPS - 1:
                        # fold acc halves (DVE fp16 2x), partition-reduce via
                        # a ones-matmul, approx-reciprocal, normalize
                        acc2 = st2["acc"]
                        foldd = fold_pool.tile([P, QBLK], f16, tag="foldd")
                        nc.vector.tensor_add(
                            foldd[:], acc2[:, :QBLK], acc2[:, QBLK:]
                        )
                        sums_ps = s_psum.tile([P, QBLK], f32, tag="sums")
                        nc.tensor.matmul(
                            sums_ps[:], ones[:], foldd[:], start=True, stop=True
                        )
                        recip = out_pool.tile([P, QBLK], f32, tag="recip")
                        nc.vector.reciprocal_approx_fast(recip[:], sums_ps[:])
                        o_sb = out_pool.tile([P, QBLK], f32, tag="osb")
                        nc.vector.tensor_mul(o_sb[:], out_ps[:], recip[:])
                        q_sl2 = slice(qb2 * QBLK, (qb2 + 1) * QBLK)
                        nc.sync.dma_start(outT[pair2][:, q_sl2], o_sb[:])
                        del state[key2]
                        if qb2 == N_QB - 1:
                            tiles.pop(pair2 - 1, None)

    import concourse.mybir as mybir2

    # Raw bass never runs the extended-inst encoding pass; without it the
    # NEFF compiler sees empty .instr on InstISA -> "ISA wrong length".
    mybir2.codegen_inst_isa_subclasses(nc)
    _split_excess_waits(nc)
    return nc


def _get_compiled():
    global _COMPILED
    if _COMPILED is None:
        _COMPILED = _build()
    return _COMPILED


def _round_f32r(x):
    """Round fp32 to the fp32r grid: round-to-nearest-even at 11 mantissa
    bits (verified bit-exact against the on-chip DVE fp32->fp32r cast)."""
    b = np.ascontiguousarray(x, dtype=np.float32).view(np.uint32).astype(np.uint64)
    drop = np.uint64(12)
    half = np.uint64(1 << 11)
    lsb = (b >> drop) & np.uint64(1)
    r = (b + half - np.uint64(1) + lsb) & np.uint64(0xFFFFF000)
    return r.astype(np.uint32).view(np.float32).reshape(x.shape)


def _shard_inputs(query, key, value):
    """Full [B,S,H,D] inputs -> per-core input maps (host-side Ulysses)."""
    qT_all = np.ascontiguousarray(np.transpose(query, (0, 2, 3, 1))).reshape(
        B * H, D, S
    )
    kT_all = np.ascontiguousarray(np.transpose(key, (0, 2, 3, 1))).reshape(
        B * H, D, S
    )
    v_all = np.ascontiguousarray(np.transpose(value, (0, 2, 1, 3))).reshape(
        B * H, S, D
    )
    qT_all = qT_all * np.float32(SCALE * EXP_GAMMA / 16.0)
    in_maps = []
    for c in range(N_CORES):
        sl = slice(c * PAIRS_PER_CORE, (c + 1) * PAIRS_PER_CORE)
        in_maps.append(
            {
                "qT": _round_f32r(qT_all[sl]),
                "kT": _round_f32r(kT_all[sl]),
                "v": np.ascontiguousarray(v_all[sl]).astype(np.float16),
            }
        )
    return in_maps


def _gather_output(results):
    outT_all = np.concatenate([r["outT"] for r in results], axis=0)  # [BH, D, S]
    out = outT_all.reshape(B, H, D, S).transpose(0, 3, 1, 2)  # [B, S, H, D]
    return np.ascontiguousarray(out)


def kernel(query, key, value, _run_kwargs=None):
    from concourse.bass_utils import run_bass_kernel_spmd

    nc = _get_compiled()
    in_maps = _shard_inputs(
        np.asarray(query, dtype=np.float32),
        np.asarray(key, dtype=np.float32),
        np.asarray(value, dtype=np.float32),
    )
    kwargs = _run_kwargs or {}
    res = run_bass_kernel_spmd(nc, in_maps, core_ids=list(range(N_CORES)), **kwargs)
    out = _gather_output(res.results)
    if _run_kwargs is not None:
        kernel.last_result = res
    return out


# revision 17
# speedup vs baseline: 1.0322x; 1.0322x over previous
"""Multi-head attention (B=2, S=2048, H=16, D=128, fp32, non-causal) on 8
Trainium2 NeuronCores.

Strategy: the 32 (batch, head) pairs are independent -> head-parallel
(Ulysses-style) sharding, 4 pairs per core, no on-device collectives.
The host pre-transposes Q and K to [d, s] layout per pair (so the
contraction dim d lands on SBUF partitions with no on-chip transposes),
and the kernel produces out^T [d, s] which the host transposes back.

Per pair the kernel computes scores^T = K @ Q^T tile-by-tile on the PE
(softmax's reduction dim sk lands on partitions). Q is pre-scaled by
softmax_scale/16 on the host so the PSUM scores arrive as t = s*scale/16;
exp(16t) is then computed on TWO engines in parallel:
  - ACT: activation Exp with scale=16 (most groups)
  - DVE: two custom ops: p = deg-4 minimax poly(t) ~ e^t, then p^16 via
    four squarings (exact to ~9e-4 rel over the full score range)
The per-column exp sums accumulate on the DVE and GPSIMD(Pool) engines
(fp16 2x adds), fold to [128,512], partition-reduce via a ones-matmul,
reciprocal via the 1-instruction DVE RECIPROCAL_APPROX_FAST, and a DVE
multiply normalizes the PV accumulation from PSUM.

The PE instruction stream is a flat software pipeline across all
(pair, qblock, group) work items with PV lagging QK by LAG groups, so
the PE never waits on exp latency and stays in its max p-state.
"""

import math

import numpy as np

B, S, H, D = 2, 2048, 16, 128
N_CORES = 8
PAIRS_PER_CORE = (B * H) // N_CORES  # 4
P = 128
QBLK = 512  # q columns per q-block (one PSUM bank of fp32)
N_QB = S // QBLK  # 4
N_SK = S // P  # 16 sk tiles per pair
SK_PER_GROUP = 2  # sk tiles per scores/exp group ([128, 1024] psum tiles)
N_GROUPS = N_SK // SK_PER_GROUP  # 8
GW = SK_PER_GROUP * QBLK  # group width: 1024
SCALE = 1.0 / math.sqrt(D)

# deg-4 minimax fit of e^t on [-0.62, 0.62] with p(0)=1 pinned, factored as
# m*(u^2+Q1B*u+Q1C)(u^2+Q2B*u+1) in u = GAMMA*t; the unit constant term of
# the second quadratic keeps the DVE op to 3 scalars + the One hw const
# (the Src1-carrying encoding faults on this walrus), and m folds into the
# first squaring of the ^16 op. Max rel err of p^16 vs e^(16t): 1.3e-3
# (scores*scale peak at 8.92 -> t=0.56).
EXP_GAMMA = 0.5040394520163475
EXP_Q1B = 0.2899882226460399
EXP_Q1C = 1.5697056417858954
EXP_Q2B = 1.7986497690988466
EXP_M = 0.6370621174950181
ACT_SCALE = 16.0 / EXP_GAMMA   # scores arrive as s*SCALE*GAMMA/16

LAG = 3  # PV trails QK by this many groups in the PE stream

# exp-engine assignment: DVE takes these groups of each qblock, ACT the rest.
# The GPSIMD(Pool) engine is kept OFF the hot path: its tensor ops run 4-7x
# slower than DVE and stall concurrent DVE ops on the shared SBUF write port
# (measured: DVE adds collide up to 3.6us while Pool runs), and GPSIMD cannot
# access PSUM at all.
DVE_EXP_GROUPS = (0,)

_COMPILED = None
_CUSTOM_OPS = None


def _patch_tile_drain():
    """Workaround for walrus 'Too many sync wait commands' on the TileContext
    tail Drain: redistribute all but one of the drain's sem waits onto
    single-wait NoOps on the sync engine (program order places them after the
    drain and before the all-engine barrier, which preserves semantics)."""
    import concourse.mybir as mybir
    import concourse.tile as tile
    from concourse.vector_clock import ScopedClock

    if getattr(tile.TileContext, "_ant_drain_patched", False):
        return

    def _drain_and_barrier(self, tick_clock, wait_clock):
        drain_inst = self.nc.sync.drain()
        wait_clock.add_sem_waits(
            drain_inst.ins, ScopedClock({None: tick_clock.global_clock})
        )
        si = drain_inst.ins.sync_info
        if si is not None and si.on_wait and len(si.on_wait) > 1:
            waits = list(si.on_wait)
            si.on_wait = waits[:1]
            engines = [
                self.nc.sync, self.nc.vector, self.nc.scalar,
                self.nc.tensor, self.nc.gpsimd,
            ]
            for i, w in enumerate(waits[1:]):
                nop = engines[i % len(engines)].nop(nofuse=True)
                nop.ins.sync_info = mybir.SyncInfo(on_wait=[w], on_update=[])

        self.nc.all_engine_barrier()
        assert self.sems is not None
        popped = self.nc._tile_sem_poison_stack.pop()
        assert popped is self._sem_poison
        self.nc.clear_and_free_semaphores(list(self.sems.allocated().values()))
        self.nc.all_engine_barrier()

    tile.TileContext._drain_and_barrier = _drain_and_barrier
    tile.TileContext._ant_drain_patched = True


def _split_excess_waits(nc):
    """This container's walrus rejects instructions carrying more than a
    struct-dependent number of semaphore waits (setupSyncWait: 'Too many
    sync wait commands'): 1 for Matmult/Ldweights (S3_LW struct), 2 for
    everything else. Hoist the excess onto NoOps inserted just before the
    instruction on the same engine — same-engine program order guarantees
    they are honored before the instruction issues."""
    import concourse.mybir as mybir

    seq = 0
    for f in nc.m.functions:
        for b in f.blocks:
            insts = list(b.instructions)
            out = []
            changed = False
            for inst in insts:
                max_waits = 1
                si = inst.sync_info
                if si is not None and si.on_wait and len(si.on_wait) > max_waits:
                    waits = list(si.on_wait)
                    si.on_wait = waits[:max_waits]
                    for w in waits[max_waits:]:
                        nop = mybir.InstNoOp(name=f"ant-waitsplit-{seq}")
                        seq += 1
                        nop.engine = inst.engine
                        nop.sync_info = mybir.SyncInfo(
                            on_wait=[w], on_update=[]
                        )
                        out.append(nop)
                    changed = True
                out.append(inst)
            if changed:
                b.instructions = out
    return nc


def _get_custom_ops():
    """Register two custom DVE ops at runtime (self-pinning their uop shas):
    EXP16_P4_ANT: p = 1 + t*((c1 + c2*t) + t^2*(c3 + c4*t)), c4 via Src1.
    EXP16_SQ4_ANT: p^16 via four squarings."""
    global _CUSTOM_OPS
    if _CUSTOM_OPS is not None:
        return _CUSTOM_OPS
    import concourse.dve_ops as dve_ops
    from concourse.dve_ops import DveOp
    from concourse.dve_spec import (
        C0, C1, C2, One, Spec, Src0, Src1, _has_src1, lower, sq,
    )
    from concourse.dve_uop import DveOpSpec

    def register(name, spec):
        if name not in dve_ops._SUB_OPCODE_FOR_NAME:
            row = max(dve_ops._SUB_OPCODE_FOR_NAME.values()) + 1
            assert row < 0x20, "custom-DVE opcode rows exhausted"
            dve_ops._SUB_OPCODE_FOR_NAME[name] = row
        row = dve_ops._SUB_OPCODE_FOR_NAME[name]
        uops = lower(spec, ver="v3")
        sha = DveOpSpec(
            name=name, opcode=row, uops=uops, rd1_en=_has_src1(spec)
        ).sha("v3")
        op = DveOp(name, spec, subdim=False, uops_sha={"v3": sha})
        existing = [o for o in dve_ops.OPS if o.name == name]
        if not existing:
            dve_ops.OPS.append(op)
            return op
        return existing[0]

    u = Src0
    u2 = sq(u)
    poly = ((u2 + u * C0) + C1) * ((u2 + u * C2) + One)
    exp_p4 = register(
        "EXP16_P4B_ANT",
        Spec(
            body=poly,
            reference=lambda in0, in1, s0, s1, imm2: (
                in0 * in0 + s0 * in0 + s1
            ) * (in0 * in0 + imm2 * in0 + 1.0),
        ),
    )
    sq4 = register(
        "EXP16_SQ4B_ANT",
        Spec(
            body=sq(sq(sq(sq(Src0 * C0)))),
            reference=lambda in0, in1, s0, s1, imm2: (s0 * in0) ** 16,
        ),
    )
    _CUSTOM_OPS = (exp_p4, sq4)
    return _CUSTOM_OPS


def _build():
    import concourse.bass as bass
    import concourse.mybir as mybir
    import concourse.tile as tile

    _patch_tile_drain()
    exp_p4, sq4 = _get_custom_ops()

    f32 = mybir.dt.float32
    f32r = mybir.dt.float32r
    f16 = mybir.dt.float16
    nc = bass.Bass()

    # Q arrives pre-scaled by SCALE/16 and pre-rounded to the fp32r grid;
    # K pre-rounded to fp32r; V pre-cast to fp16.
    qT = nc.dram_tensor("qT", [PAIRS_PER_CORE, P, S], f32r, kind="ExternalInput")
    kT = nc.dram_tensor("kT", [PAIRS_PER_CORE, P, S], f32r, kind="ExternalInput")
    v = nc.dram_tensor("v", [PAIRS_PER_CORE, S, D], f16, kind="ExternalInput")
    outT = nc.dram_tensor("outT", [PAIRS_PER_CORE, P, S], f32, kind="ExternalOutput")

    TOT = PAIRS_PER_CORE * N_QB * N_GROUPS  # flat group count

    def decode(G):
        return G // (N_QB * N_GROUPS), (G // N_GROUPS) % N_QB, G % N_GROUPS

    with tile.TileContext(nc) as tc:
        with (
            tc.tile_pool(name="const", bufs=1) as const_pool,
            tc.tile_pool(name="inp", bufs=2) as inp_pool,
            tc.tile_pool(name="exp", bufs=8) as exp_pool,
            tc.tile_pool(name="tmp", bufs=2) as tmp_pool,
            tc.tile_pool(name="accd", bufs=2) as accd_pool,
            tc.tile_pool(name="fold", bufs=2) as fold_pool,
            tc.tile_pool(name="outsb", bufs=4) as out_pool,
            tc.tile_pool(name="sc_ps", bufs=2, space="PSUM") as sc_psum,
            tc.tile_pool(name="o_ps", bufs=2, space="PSUM") as o_psum,
            tc.tile_pool(name="s_ps", bufs=2, space="PSUM") as s_psum,
        ):
            ones = const_pool.tile([P, P], f16)
            nc.vector.memset(ones[:], 1.0)

            def emit_loads(pair):
                qT_sb = inp_pool.tile([P, S], f32r, tag="qT")
                kT_sb = inp_pool.tile([P, S], f32r, tag="kT")
                v_sb = inp_pool.tile([P, N_SK, D], f16, tag="v")
                nQ = 4
                sl0 = slice(0, S // nQ)
                # order matters: the DMA queue is FIFO and the first QK only
                # needs kT/qT chunk 0, while the first PV (LAG groups later)
                # needs V -- queue V right after the first chunks
                nc.sync.dma_start(kT_sb[:, sl0], kT[pair][:, sl0])
                nc.sync.dma_start(qT_sb[:, sl0], qT[pair][:, sl0])
                nc.sync.dma_start(
                    v_sb[:], v[pair].rearrange("(t p) d -> p t d", p=P)
                )
                for h in range(1, nQ):
                    sl = slice(h * (S // nQ), (h + 1) * (S // nQ))
                    nc.sync.dma_start(kT_sb[:, sl], kT[pair][:, sl])
                rest = slice(S // nQ, S)
                nc.sync.dma_start(qT_sb[:, rest], qT[pair][:, rest])
                return qT_sb, kT_sb, v_sb

            tiles = {0: emit_loads(0)}
            # per-live-qblock state: (out_ps, accd, accp, e_tiles, sums_ps...)
            state = {}

            for G in range(TOT + LAG):
                if G < TOT:
                    pair, qb, g = decode(G)
                    if g == 0 and qb == 0 and pair + 1 < PAIRS_PER_CORE:
                        tiles[pair + 1] = emit_loads(pair + 1)
                    qT_sb, kT_sb, v_sb = tiles[pair]
                    key = (pair, qb)
                    if g == 0:
                        state[key] = {
                            "out_ps": o_psum.tile(
                                [P, QBLK], f32, tag="ops", name="out_ps"
                            ),
                            "acc": accd_pool.tile(
                                [P, GW], f16, tag="acc", name="acc"
                            ),
                            "e": [None] * N_GROUPS,
                        }
                    st = state[key]
                    q_sl = slice(qb * QBLK, (qb + 1) * QBLK)

                    # scores^T for 2 sk tiles
                    sc = sc_psum.tile([P, GW], f32, tag="sc")
                    for j in range(SK_PER_GROUP):
                        sk = g * SK_PER_GROUP + j
                        nc.tensor.matmul(
                            sc[:, j * QBLK : (j + 1) * QBLK],
                            kT_sb[:, sk * P : (sk + 1) * P],
                            qT_sb[:, q_sl],
                            start=True,
                            stop=True,
                        )
                    # exp on ACT or DVE
                    e = exp_pool.tile([P, GW], f16, tag="e")
                    st["e"][g] = e
                    if g in DVE_EXP_GROUPS:
                        tmp = tmp_pool.tile([P, GW], f32, tag="tmp")
                        nc.vector._custom_dve(
                            exp_p4, out=tmp[:], in0=sc[:],
                            s0=EXP_Q1B, s1=EXP_Q1C, imm2=EXP_Q2B,
                        )
                        nc.vector._custom_dve(sq4, out=e[:], in0=tmp[:], s0=EXP_M)
                    else:
                        nc.scalar.activation(
                            e[:], sc[:], mybir.ActivationFunctionType.Exp,
                            scale=ACT_SCALE,
                        )
                    # exp-sum accumulation on DVE (fp16 2x); the first add
                    # at g=1 combines groups 0 and 1, saving the init copy
                    acc = st["acc"]
                    if g == 1:
                        nc.vector.tensor_add(acc[:], st["e"][0][:], e[:])
                    elif g >= 2:
                        nc.vector.tensor_add(acc[:], acc[:], e[:])

                if G - LAG >= 0:
                    pair2, qb2, g2 = decode(G - LAG)
                    key2 = (pair2, qb2)
                    st2 = state[key2]
                    v_sb2 = tiles[pair2][2]
                    ep = st2["e"][g2]
                    out_ps = st2["out_ps"]
                    for j in range(SK_PER_GROUP):
                        sk = g2 * SK_PER_GROUP + j
                        nc.tensor.matmul(
                            out_ps[:],
                            v_sb2[:, sk, :],
                            ep[:, j * QBLK : (j + 1) * QBLK],
                            start=(sk == 0),
                            stop=(sk == N_SK - 1),
                        )
                    if g2 == N_GROUPS - 1:
                        # fold acc halves (DVE fp16 2x), partition-reduce via
                        # a ones-matmul, approx-reciprocal, normalize
                        acc2 = st2["acc"]
                        foldd = fold_pool.tile([P, QBLK], f16, tag="foldd")
                        nc.vector.tensor_add(
                            foldd[:], acc2[:, :QBLK], acc2[:, QBLK:]
                        )
                        sums_ps = s_psum.tile([P, QBLK], f32, tag="sums")
                        nc.tensor.matmul(
                            sums_ps[:], ones[:], foldd[:], start=True, stop=True
                        )
                        recip = out_pool.tile([P, QBLK], f32, tag="recip")
                        nc.vector.reciprocal_approx_fast(recip[:], sums_ps[:])
                        o_sb = out_pool.tile([P, QBLK], f32, tag="osb")
                        nc.vector.tensor_mul(o_sb[:], out_ps[:], recip[:])
                        q_sl2 = slice(qb2 * QBLK, (qb2 + 1) * QBLK)
                        nc.sync.dma_start(outT[pair2][:, q_sl2], o_sb[:])
                        del state[key2]
                        if qb2 == N_QB - 1:
                            tiles.pop(pair2 - 1, None)

    import concourse.mybir as mybir2

    # Raw bass never runs the extended-inst encoding pass; without it the
    # NEFF compiler sees empty .instr on InstISA -> "ISA wrong length".
    mybir2.codegen_inst_isa_subclasses(nc)
    _split_excess_waits(nc)
    return nc


def _get_compiled():
    global _COMPILED
    if _COMPILED is None:
        _COMPILED = _build()
    return _COMPILED


def _round_f32r(x):
    """Round fp32 to the fp32r grid: round-to-nearest-even at 11 mantissa
    bits (verified bit-exact against the on-chip DVE fp32->fp32r cast)."""
    b = np.ascontiguousarray(x, dtype=np.float32).view(np.uint32).astype(np.uint64)
    drop = np.uint64(12)
    half = np.uint64(1 << 11)
    lsb = (b >> drop) & np.uint64(1)
    r = (b + half - np.uint64(1) + lsb) & np.uint64(0xFFFFF000)
    return r.astype(np.uint32).view(np.float32).reshape(x.shape)


def _shard_inputs(query, key, value):
    """Full [B,S,H,D] inputs -> per-core input maps (host-side Ulysses)."""
    qT_all = np.ascontiguousarray(np.transpose(query, (0, 2, 3, 1))).reshape(
        B * H, D, S
    )
    kT_all = np.ascontiguousarray(np.transpose(key, (0, 2, 3, 1))).reshape(
        B * H, D, S
    )
    v_all = np.ascontiguousarray(np.transpose(value, (0, 2, 1, 3))).reshape(
        B * H, S, D
    )
    qT_all = qT_all * np.float32(SCALE * EXP_GAMMA / 16.0)
    in_maps = []
    for c in range(N_CORES):
        sl = slice(c * PAIRS_PER_CORE, (c + 1) * PAIRS_PER_CORE)
        in_maps.append(
            {
                "qT": _round_f32r(qT_all[sl]),
                "kT": _round_f32r(kT_all[sl]),
                "v": np.ascontiguousarray(v_all[sl]).astype(np.float16),
            }
        )
    return in_maps


def _gather_output(results):
    outT_all = np.concatenate([r["outT"] for r in results], axis=0)  # [BH, D, S]
    out = outT_all.reshape(B, H, D, S).transpose(0, 3, 1, 2)  # [B, S, H, D]
    return np.ascontiguousarray(out)


def kernel(query, key, value, _run_kwargs=None):
    from concourse.bass_utils import run_bass_kernel_spmd

    nc = _get_compiled()
    in_maps = _shard_inputs(
        np.asarray(query, dtype=np.float32),
        np.asarray(key, dtype=np.float32),
        np.asarray(value, dtype=np.float32),
    )
    kwargs = _run_kwargs or {}
    res = run_bass_kernel_spmd(nc, in_maps, core_ids=list(range(N_CORES)), **kwargs)
    out = _gather_output(res.results)
    if _run_kwargs is not None:
        kernel.last_result = res
    return out
